# revision 9
# baseline (speedup 1.0000x reference)
"""Trainium2 Bass kernel for the DynamicInnerLoop problem.

Algorithm (exact algebraic collapse of the reference scan):
  The reference runs 10 steps; each step evaluates two 3-layer MLPs whose
  first layer is a 128x(N+64) GEMV against concat(params, enc).  Since
  params only changes by scalar multiples of `gradients`
  (params_{t+1} = params_t - m_t*ss_t*gradients), the first-layer
  pre-activations evolve as  a_{t+1} = a_t - m_t*ss_t*gv  where
  a_0 = W1[:, :N]@p0 + W1[:, N:]@enc + b1  and  gv = W1[:, :N]@gradients.
  So the big weights are read exactly once, and the 10-step loop runs on
  128-dim vectors.  Finally params_out = p0 - 0.1*S*gradients with
  S = sum_t m_t*ss_t, and count = sum_t m_t.

Sharding: columns of both W1 matrices are split across 8 cores (25000
each, zero-padded to 25088 = 196*128).  Each core computes partial
(a0, gv) for both MLPs with x-stationary matmuls, the 2KB partials are
AllGathered, reduced with one selection matmul, and the tiny recurrence
runs replicated on every core; each core then updates its own shard of
params.
"""

import numpy as np

import concourse.bass as bass
import concourse.bacc as bacc
import concourse.tile as tile
from concourse import mybir
from concourse.bass_utils import run_bass_kernel_spmd

# ---------------------------------------------------------------- constants
NCORES = 8
N = 200000
NS = N // NCORES            # 25000 params per core
P = 128                     # partition dim / W1 row count
C = NS // P + (1 if NS % P else 0)  # 196 chunks of 128
NSP = C * P                 # 25088 padded shard length
MAX_STEPS = 10
CTX = 100

# W stream tiling: combined (sp|st) chunks are 256 wide; DMA tiles hold
# CHUNKS_PER_TILE of them.
WCOLS = C * 2 * P           # 50176 columns of the combined wt tensor
CHUNKS_PER_TILE = 7
TILE_COLS = CHUNKS_PER_TILE * 2 * P   # 1792
NDMA = C // CHUNKS_PER_TILE           # 28 DMA tiles

F32 = mybir.dt.float32
I32 = mybir.dt.int32

_COMPILED = None  # cached (nc, meta)


def _build_program():
    nc = bacc.Bacc(
        "TRN2",
        target_bir_lowering=False,
        debug=False,
        num_devices=NCORES,
    )

    # ------------------------------------------------------------- I/O
    wt = nc.dram_tensor("wt", [P, WCOLS], F32, kind="ExternalInput")
    x_mm = nc.dram_tensor("x_mm", [P, C, 2], F32, kind="ExternalInput")
    x_pg = nc.dram_tensor("x_pg", [P, C, 2], F32, kind="ExternalInput")
    tail = nc.dram_tensor("tail", [P // 2 + 1, 2 * P], F32, kind="ExternalInput")
    ctx_in = nc.dram_tensor("ctx", [CTX, 1], F32, kind="ExternalInput")
    cw1t = nc.dram_tensor("cw1t", [CTX, 64], F32, kind="ExternalInput")
    cb1 = nc.dram_tensor("cb1", [64, 1], F32, kind="ExternalInput")
    cw2t = nc.dram_tensor("cw2t", [64, 64], F32, kind="ExternalInput")
    cb2 = nc.dram_tensor("cb2", [64, 1], F32, kind="ExternalInput")
    w2cat = nc.dram_tensor("w2cat", [P, P], F32, kind="ExternalInput")
    b2cat = nc.dram_tensor("b2cat", [P, 1], F32, kind="ExternalInput")
    w3cat = nc.dram_tensor("w3cat", [P, 1], F32, kind="ExternalInput")
    b3row = nc.dram_tensor("b3row", [1, 2], F32, kind="ExternalInput")
    rvrow = nc.dram_tensor("rvrow", [1, MAX_STEPS], F32, kind="ExternalInput")
    sel_in = nc.dram_tensor("sel", [4 * NCORES, 4], F32, kind="ExternalInput")

    out_params = nc.dram_tensor("out_params", [P, C], F32, kind="ExternalOutput")
    out_count = nc.dram_tensor("out_count", [1, 1], I32, kind="ExternalOutput")

    with tile.TileContext(nc) as tc:
        with (
            tc.tile_pool(name="wpool", bufs=6) as wpool,
            tc.tile_pool(name="sbuf", bufs=1) as sb,
            tc.tile_pool(name="psum_gemv", bufs=1, space="PSUM") as pg,
            tc.tile_pool(name="psum_small", bufs=3, space="PSUM") as ps,
            tc.tile_pool(name="psum_hold", bufs=1, space="PSUM") as ph,
            tc.tile_pool(name="dram", bufs=1, space="DRAM") as dram,
        ):
            # ----------------------------------------------- small loads
            x_mm_t = sb.tile([P, C, 2], F32)
            x_pg_t = sb.tile([P, C, 2], F32)
            tail_t = sb.tile([P // 2 + 1, 2 * P], F32)
            ctx_t = sb.tile([CTX, 1], F32)
            cw1t_t = sb.tile([CTX, 64], F32)
            cb1_t = sb.tile([64, 1], F32)
            cw2t_t = sb.tile([64, 64], F32)
            cb2_t = sb.tile([64, 1], F32)
            w2cat_t = sb.tile([P, P], F32)
            b2cat_t = sb.tile([P, 1], F32)
            w3cat_t = sb.tile([P, 1], F32)
            b3row_t = sb.tile([1, 2], F32)
            rvrow_t = sb.tile([1, MAX_STEPS], F32)
            sel_t = sb.tile([4 * NCORES, 4], F32)

            nc.sync.dma_start(x_mm_t[:], x_mm[:])
            nc.sync.dma_start(x_pg_t[:], x_pg[:])
            nc.sync.dma_start(tail_t[:], tail[:])
            nc.sync.dma_start(ctx_t[:], ctx_in[:])
            nc.sync.dma_start(cw1t_t[:], cw1t[:])
            nc.sync.dma_start(cb1_t[:], cb1[:])
            nc.sync.dma_start(cw2t_t[:], cw2t[:])
            nc.sync.dma_start(cb2_t[:], cb2[:])
            nc.sync.dma_start(w2cat_t[:], w2cat[:])
            nc.sync.dma_start(b2cat_t[:], b2cat[:])
            nc.sync.dma_start(w3cat_t[:], w3cat[:])
            nc.sync.dma_start(b3row_t[:], b3row[:])
            nc.sync.dma_start(rvrow_t[:], rvrow[:])
            nc.sync.dma_start(sel_t[:], sel_in[:])

            # ----------------------------------------------- constants
            enc2 = sb.tile([P // 2 + 1, 2], F32)      # [enc;1 | 0]
            nc.vector.memset(enc2[:], 0.0)
            nc.vector.memset(enc2[64:65, 0:1], 1.0)

            h2z = sb.tile([P, 2], F32)                # zero-masked layer2 act
            nc.vector.memset(h2z[:], 0.0)
            zcol = sb.tile([P, 1], F32)               # all-zero column
            nc.vector.memset(zcol[:], 0.0)

            act_hist = sb.tile([1, MAX_STEPS + 1], F32)
            nc.vector.memset(act_hist[:], 0.0)
            nc.vector.memset(act_hist[:, 0:1], 1.0)

            ss_hist = sb.tile([1, MAX_STEPS], F32)
            neg01 = sb.tile([1, P], F32)
            nc.vector.memset(neg01[:], -0.1)

            # ----------------------------------------------- context encoder
            # enc = ce_w2 @ relu(ce_w1 @ ctx + ce_b1) + ce_b2, written into
            # enc2[:64, 0].  All cores compute it; the tail input is
            # pre-divided by NCORES so the AllGather-sum is exact.
            p_ce1 = ps.tile([64, 1], F32, tag="small")
            nc.tensor.matmul(p_ce1[:], cw1t_t[:], ctx_t[:], start=True, stop=True)
            s1 = sb.tile([64, 1], F32)
            nc.scalar.activation(
                s1[:], p_ce1[:], mybir.ActivationFunctionType.Relu, bias=cb1_t[:]
            )
            p_ce2 = ps.tile([64, 1], F32, tag="small")
            nc.tensor.matmul(p_ce2[:], cw2t_t[:], s1[:], start=True, stop=True)
            nc.scalar.activation(
                enc2[0:64, 0:1], p_ce2[:], mybir.ActivationFunctionType.Identity,
                bias=cb2_t[:],
            )

            # ----------------------------------------------- big GEMV stream
            # psum_cat[j, s*128+n] accumulates (W_s @ x_j)[n] partials.
            psum_cat = pg.tile([2, 2 * P], F32)
            for d in range(NDMA):
                wt_t = wpool.tile([P, TILE_COLS], F32)
                nc.sync.dma_start(
                    wt_t[:], wt[:, d * TILE_COLS:(d + 1) * TILE_COLS]
                )
                for j in range(CHUNKS_PER_TILE):
                    c = d * CHUNKS_PER_TILE + j
                    nc.tensor.matmul(
                        psum_cat[:],
                        x_mm_t[:, c, :],
                        wt_t[:, j * 2 * P:(j + 1) * 2 * P],
                        start=(c == 0),
                        stop=False,
                    )
            # e/bias contribution last (closes the group; depends on the tiny
            # ce chain, which is ready long before the last weight tile)
            nc.tensor.matmul(psum_cat[:], enc2[:], tail_t[:], start=False, stop=True)

            # ----------------------------------------------- gather partials
            cc_in = dram.tile([4, P], F32)
            cc_out = dram.tile([4 * NCORES, P], F32)
            # psum_cat rows (j, s*128+n) map to cc_in rows 2j+s.
            cat_sb = sb.tile([2, 2 * P], F32)
            nc.scalar.copy(cat_sb[:], psum_cat[:])
            nc.sync.dma_start(cc_in[:].rearrange("(j s) n -> j (s n)", j=2), cat_sb[:])
            nc.gpsimd.collective_compute(
                "AllGather",
                mybir.AluOpType.bypass,
                replica_groups=[list(range(NCORES))],
                ins=[cc_in.opt()],
                outs=[cc_out.opt()],
            )
            g_sb = sb.tile([4 * NCORES, P], F32)
            nc.sync.dma_start(g_sb[:], cc_out[:])

            # reduce 8 partials: sums[m, 2s+j] = sum_k G[k, m]*sel[k, 2s+j]
            psum_sums = ph.tile([P, 2, 2], F32)  # [(m), (s), (j: a=0,g=1)]
            nc.tensor.matmul(
                psum_sums[:].rearrange("m s j -> m (s j)"),
                g_sb[:], sel_t[:], start=True, stop=True,
            )

            a_sb = sb.tile([P, 2], F32)
            nc.scalar.copy(a_sb[:], psum_sums[:, :, 0])

            # ----------------------------------------------- 10-step loop
            for t in range(MAX_STEPS):
                u = sb.tile([P, 2], F32, tag="u")
                nc.vector.tensor_scalar_max(u[:], a_sb[:], 0.0)

                p2 = ps.tile([P, 2], F32, tag="small")
                nc.tensor.matmul(p2[:], w2cat_t[:], u[:], start=True, stop=True)

                # h2z halves: relu(p2 + b2), off-diagonal halves stay zero
                nc.vector.scalar_tensor_tensor(
                    h2z[0:64, 0:1], p2[0:64, 0:1], b2cat_t[0:64, 0:1],
                    zcol[0:64, 0:1],
                    op0=mybir.AluOpType.add, op1=mybir.AluOpType.max,
                )
                nc.vector.scalar_tensor_tensor(
                    h2z[64:128, 1:2], p2[64:128, 1:2], b2cat_t[64:128, 0:1],
                    zcol[64:128, 0:1],
                    op0=mybir.AluOpType.add, op1=mybir.AluOpType.max,
                )

                p3 = ps.tile([1, 2], F32, tag="small")
                nc.tensor.matmul(p3[:], w3cat_t[:], h2z[:], start=True, stop=True)

                t3 = sb.tile([1, 2], F32, tag="t3")
                nc.vector.tensor_tensor(t3[:], p3[:], b3row_t[:],
                                        op=mybir.AluOpType.add)
                sig = sb.tile([1, 2], F32, tag="sig")
                nc.scalar.activation(sig[:], t3[:],
                                     mybir.ActivationFunctionType.Sigmoid)

                # notstop = (r'_t >= stop_prob); r'_0 = 2.0 so step 0 never stops
                notstop = sb.tile([1, 1], F32, tag="notstop")
                nc.vector.tensor_tensor(
                    notstop[:], rvrow_t[:, t:t + 1], sig[:, 1:2],
                    op=mybir.AluOpType.is_ge,
                )
                nc.vector.tensor_tensor(
                    act_hist[:, t + 1:t + 2], act_hist[:, t:t + 1], notstop[:],
                    op=mybir.AluOpType.mult,
                )
                nc.vector.tensor_tensor(
                    ss_hist[:, t:t + 1], act_hist[:, t:t + 1], sig[:, 0:1],
                    op=mybir.AluOpType.mult,
                )

                # a -= 0.1*active*ss * gv   (broadcast via -0.1 ones matmul)
                pb = ps.tile([P, 1], F32, tag="small")
                nc.tensor.matmul(pb[:], neg01[:], ss_hist[:, t:t + 1],
                                 start=True, stop=True)
                nc.vector.scalar_tensor_tensor(
                    a_sb[:], psum_sums[:, :, 1], pb[:], a_sb[:],
                    op0=mybir.AluOpType.mult, op1=mybir.AluOpType.add,
                )

            # ----------------------------------------------- epilogue
            s_sum = sb.tile([1, 1], F32)
            nc.vector.tensor_reduce(
                s_sum[:], ss_hist[:], axis=mybir.AxisListType.X,
                op=mybir.AluOpType.add,
            )
            pS = ps.tile([P, 1], F32, tag="small")
            nc.tensor.matmul(pS[:], neg01[:], s_sum[:], start=True, stop=True)

            out_t = sb.tile([P, C], F32)
            nc.vector.scalar_tensor_tensor(
                out_t[:], x_pg_t[:, :, 1], pS[:], x_pg_t[:, :, 0],
                op0=mybir.AluOpType.mult, op1=mybir.AluOpType.add,
            )
            nc.sync.dma_start(out_params[:], out_t[:])

            cnt_f = sb.tile([1, 1], F32)
            nc.vector.tensor_reduce(
                cnt_f[:], act_hist[:, 0:MAX_STEPS], axis=mybir.AxisListType.X,
                op=mybir.AluOpType.add,
            )
            cnt_i = sb.tile([1, 1], I32)
            nc.vector.tensor_copy(cnt_i[:], cnt_f[:])
            nc.sync.dma_start(out_count[:], cnt_i[:])

    nc.compile()
    return nc


def _get_program():
    global _COMPILED
    if _COMPILED is None:
        _COMPILED = _build_program()
    return _COMPILED


def _prep_inputs(inputs):
    """Build the 8 per-core input maps from full-size numpy inputs."""
    f = lambda k: np.asarray(inputs[k], dtype=np.float32)
    p0, g = f("initial_params"), f("gradients")
    sp_w1, st_w1 = f("sp_w1"), f("st_w1")

    p0_pad = np.zeros(NSP * NCORES, np.float32)
    p0_pad_v = p0_pad.reshape(NCORES, NSP)
    g_pad = np.zeros(NSP * NCORES, np.float32)
    g_pad_v = g_pad.reshape(NCORES, NSP)
    for r in range(NCORES):
        p0_pad_v[r, :NS] = p0[r * NS:(r + 1) * NS]
        g_pad_v[r, :NS] = g[r * NS:(r + 1) * NS]

    # shared small tensors
    tail_sp = np.concatenate(
        [sp_w1[:, N:].T, f("sp_b1")[None, :]], axis=0) / NCORES  # [65,128]
    tail_st = np.concatenate(
        [st_w1[:, N:].T, f("st_b1")[None, :]], axis=0) / NCORES
    tail = np.ascontiguousarray(
        np.concatenate([tail_sp, tail_st], axis=1), np.float32)  # [65,256]

    w2cat = np.ascontiguousarray(
        np.concatenate([f("sp_w2").T, f("st_w2").T], axis=1), np.float32)
    b2cat = np.concatenate([f("sp_b2"), f("st_b2")])[:, None].astype(np.float32)
    w3cat = np.concatenate(
        [f("sp_w3")[0], f("st_w3")[0]])[:, None].astype(np.float32)
    b3row = np.array([[f("sp_b3")[0], f("st_b3")[0]]], np.float32)
    rv = f("rand_vals").copy()
    rv[0] = 2.0  # MIN_STEPS=1: step 0 can never stop
    rvrow = rv[None, :].astype(np.float32)
    # cc_in rows per rank are (2j+s); sums column n=2s+j selects row 2j+s
    sel = np.zeros((4 * NCORES, 4), np.float32)
    for k in range(4 * NCORES):
        j, s = (k % 4) // 2, (k % 4) % 2
        sel[k, 2 * s + j] = 1.0

    shared = {
        "tail": tail,
        "ctx": f("context")[:, None].astype(np.float32),
        "cw1t": np.ascontiguousarray(f("ce_w1").T),
        "cb1": f("ce_b1")[:, None].astype(np.float32),
        "cw2t": np.ascontiguousarray(f("ce_w2").T),
        "cb2": f("ce_b2")[:, None].astype(np.float32),
        "w2cat": w2cat, "b2cat": b2cat, "w3cat": w3cat, "b3row": b3row,
        "rvrow": rvrow, "sel": sel,
    }

    in_maps = []
    for r in range(NCORES):
        sl = slice(r * NS, (r + 1) * NS)
        # wt[k, (c, s, m)] = W_s[m, r*NS + c*128 + k]
        ws = np.zeros((2, P, NSP), np.float32)
        ws[0, :, :NS] = sp_w1[:, sl]
        ws[1, :, :NS] = st_w1[:, sl]
        # [s, m, (c k)] -> [k, c, s, m]
        wt = np.ascontiguousarray(
            ws.reshape(2, P, C, P).transpose(3, 2, 0, 1)).reshape(P, WCOLS)

        x = np.stack(
            [p0_pad_v[r].reshape(C, P).T, g_pad_v[r].reshape(C, P).T],
            axis=2).astype(np.float32)  # [128, 196, 2]
        x = np.ascontiguousarray(x)

        m = dict(shared)
        m["wt"] = wt
        m["x_mm"] = x
        m["x_pg"] = x
        in_maps.append(m)
    return in_maps


LAST_RESULTS = None  # BassKernelResults of the most recent run (for test.py)


def kernel(trace=False, **inputs):
    global LAST_RESULTS
    nc = _get_program()
    in_maps = _prep_inputs(inputs)
    res = run_bass_kernel_spmd(
        nc, in_maps, list(range(NCORES)), trace=trace,
    )
    LAST_RESULTS = res
    outs = res.results
    params = np.empty(N, np.float32)
    for r in range(NCORES):
        shard = outs[r]["out_params"]  # [128, 196], (k, c) = p[c*128+k]
        params[r * NS:(r + 1) * NS] = shard.T.reshape(-1)[:NS]
    count = np.int32(outs[0]["out_count"].reshape(-1)[0])
    return params, count


# revision 23
# speedup vs baseline: 1.7898x; 1.7898x over previous
"""Trainium2 Bass kernel for the DynamicInnerLoop problem.

Algorithm (exact algebraic collapse of the reference scan):
  The reference runs 10 steps; each step evaluates two 3-layer MLPs whose
  first layer is a 128x(N+64) GEMV against concat(params, enc).  Since
  params only changes by scalar multiples of `gradients`
  (params_{t+1} = params_t - m_t*ss_t*gradients), the first-layer
  pre-activations evolve as  a_{t+1} = a_t - m_t*ss_t*gv  where
  a_0 = W1[:, :N]@p0 + W1[:, N:]@enc + b1  and  gv = W1[:, :N]@gradients.
  So the big weights are read exactly once, and the 10-step loop runs on
  128-dim vectors.  Further, because an inactive step freezes params, the
  active-gating can be moved OUT of the loop: run the recurrence
  unconditionally (a += -0.1*ss_t*gv), record the per-step logits, then
  compute stop flags / the active mask / S with a handful of vectorized
  ops.  Finally params_out = p0 - 0.1*S*gradients, count = sum(active).

Sharding: columns of both W1 matrices are split across 8 cores (25000
each, zero-padded to 25088 = 196*128).  Each core computes partial
(a0, gv) for both MLPs with x-stationary bf16 matmuls, the 2KB partials
are exchanged with a collective, and the tiny recurrence runs replicated
on every core; each core then updates its own shard of params in fp32.

The big weight stream is bf16 (halves HBM traffic and doubles PE
streaming rate); the fp32 anchors (p0, gradients, final update, a0
reduction, S) stay fp32, so the output error is ~1e-5 relative.
"""

import numpy as np
import ml_dtypes

import concourse.bass as bass
import concourse.bacc as bacc
import concourse.tile as tile
from concourse import mybir
from concourse.bass_utils import run_bass_kernel_spmd

# ---------------------------------------------------------------- constants
NCORES = 8
N = 200000
NS = N // NCORES            # 25000 params per core
P = 128
C = NS // P + (1 if NS % P else 0)  # 196 chunks of 128
NSP = C * P                 # 25088 padded shard length
MAX_STEPS = 10
CTX = 100

WCOLS = C * 2 * P           # 50176 columns of the combined bf16 wt tensor
NDMA = 2                    # two big weight DMAs -> 50 KB lines
TILE_COLS = WCOLS // NDMA   # 25088
CHUNKS_PER_TILE = C // NDMA  # 98

COLLECTIVE = "ag"           # "ag" (AllGather+reduce-mm) or "ar" (AllReduce)

F32 = mybir.dt.float32
BF16 = mybir.dt.bfloat16
I32 = mybir.dt.int32
BF = ml_dtypes.bfloat16

# packed_bf16 layout (columns)
XB = 0                      # x interleaved (c,j): 392
W2B = XB + 2 * C            # w2cat: 128
W3B = W2B + P               # w3cat: 1
TLB = W3B + 1               # tail (65 partitions used): 256
PKB_COLS = TLB + 2 * P      # 777

# packed_f32 layout (columns)
XPP = 0                     # x_pg params: 196
XPG = XPP + C               # x_pg grads: 196
B2C = XPG + C               # b2cat: 1
CW1 = B2C + 1               # ce_w1.T (100 partitions): 64
CW2 = CW1 + 64              # ce_w2.T (64 partitions): 64
CTXC = CW2 + 64             # context (100 partitions): 1
CB1C = CTXC + 1             # ce_b1 (64 partitions): 1
CB2C = CB1C + 1             # ce_b2 (64 partitions): 1
B3C = CB2C + 1              # b3 row (1 partition): 2  [sp, st]
RVC = B3C + 2               # r' row (1 partition): 10
SELA = RVC + MAX_STEPS      # (a,g)x(sp,st) selectors (32 partitions): 4
PKF_COLS = SELA + 4         # 670


def _build_program(collective=COLLECTIVE):
    nc = bacc.Bacc(
        "TRN2",
        target_bir_lowering=False,
        debug=False,
        num_devices=NCORES,
    )

    wt = nc.dram_tensor("wt", [P, WCOLS], BF16, kind="ExternalInput")
    pkb = nc.dram_tensor("pkb", [P, PKB_COLS], BF16, kind="ExternalInput")
    pkf = nc.dram_tensor("pkf", [P, PKF_COLS], F32, kind="ExternalInput")
    out_params = nc.dram_tensor("out_params", [P, C], F32, kind="ExternalOutput")
    out_count = nc.dram_tensor("out_count", [1, 1], I32, kind="ExternalOutput")

    with tile.TileContext(nc) as tc:
        with (
            tc.tile_pool(name="wpool", bufs=2) as wpool,
            tc.tile_pool(name="sbuf", bufs=1) as sb,
            tc.tile_pool(name="psum_gemv", bufs=1, space="PSUM") as pg,
            tc.tile_pool(name="psum_small", bufs=3, space="PSUM") as ps,
            tc.tile_pool(name="psum_hold", bufs=1, space="PSUM") as ph,
            tc.tile_pool(name="dram", bufs=1, space="DRAM") as dram,
        ):
            pkb_t = sb.tile([P, PKB_COLS], BF16)
            pkf_t = sb.tile([P, PKF_COLS], F32)
            nc.sync.dma_start(pkb_t[:], pkb[:])
            nc.sync.dma_start(pkf_t[:], pkf[:])

            # ----------------------------------------------- constants
            enc2 = sb.tile([P // 2 + 1, 2], BF16)     # [enc;1 | 0]
            nc.vector.memset(enc2[:], 0.0)
            nc.vector.memset(enc2[64:65, 0:1], 1.0)
            h2z = sb.tile([P, 2], BF16)               # zero-masked layer2 act
            nc.vector.memset(h2z[:], 0.0)
            zcol = sb.tile([P, 1], F32)
            nc.vector.memset(zcol[:], 0.0)
            ones_row = sb.tile([1, P], F32)
            nc.vector.memset(ones_row[:], 1.0)

            # ----------------------------------------------- context encoder
            p_ce1 = ps.tile([64, 1], F32, tag="small")
            nc.tensor.matmul(p_ce1[:], pkf_t[0:CTX, CW1:CW1 + 64],
                             pkf_t[0:CTX, CTXC:CTXC + 1], start=True, stop=True)
            s1 = sb.tile([64, 1], F32)
            nc.scalar.activation(
                s1[:], p_ce1[:], mybir.ActivationFunctionType.Relu,
                bias=pkf_t[0:64, CB1C:CB1C + 1],
            )
            p_ce2 = ps.tile([64, 1], F32, tag="small")
            nc.tensor.matmul(p_ce2[:], pkf_t[0:64, CW2:CW2 + 64], s1[:],
                             start=True, stop=True)
            nc.scalar.activation(
                enc2[0:64, 0:1], p_ce2[:], mybir.ActivationFunctionType.Identity,
                bias=pkf_t[0:64, CB2C:CB2C + 1],
            )

            # ----------------------------------------------- big GEMV stream
            psum_cat = pg.tile([2, 2 * P], F32)
            for d in range(NDMA):
                wt_t = wpool.tile([P, TILE_COLS], BF16)
                nc.sync.dma_start(
                    wt_t[:], wt[:, d * TILE_COLS:(d + 1) * TILE_COLS]
                )
                for j in range(CHUNKS_PER_TILE):
                    c = d * CHUNKS_PER_TILE + j
                    nc.tensor.matmul(
                        psum_cat[:],
                        pkb_t[:, 2 * c:2 * c + 2],
                        wt_t[:, j * 2 * P:(j + 1) * 2 * P],
                        start=(c == 0),
                        stop=False,
                    )
            # e/bias contribution closes the group
            nc.tensor.matmul(psum_cat[:], enc2[:], pkb_t[0:65, TLB:TLB + 2 * P],
                             start=False, stop=True)

            # ----------------------------------------------- exchange partials
            cat_sb = sb.tile([2, 2 * P], F32)
            nc.scalar.copy(cat_sb[:], psum_cat[:])
            cc_in = dram.tile([4, P], F32)
            # psum_cat rows (j, s*128+n) map to cc_in rows 2j+s.
            nc.sync.dma_start(
                cc_in[:].rearrange("(j s) n -> j (s n)", j=2), cat_sb[:])

            if collective == "ag":
                cc_out = dram.tile([4 * NCORES, P], F32)
                nc.gpsimd.collective_compute(
                    "AllGather",
                    mybir.AluOpType.bypass,
                    replica_groups=[list(range(NCORES))],
                    ins=[cc_in.opt()],
                    outs=[cc_out.opt()],
                )
                krows = 4 * NCORES
            else:
                cc_out = dram.tile([4, P], F32)
                nc.gpsimd.collective_compute(
                    "AllReduce",
                    mybir.AluOpType.add,
                    replica_groups=[list(range(NCORES))],
                    ins=[cc_in.opt()],
                    outs=[cc_out.opt()],
                )
                krows = 4
            g_sb = sb.tile([krows, P], F32)
            nc.sync.dma_start(g_sb[:], cc_out[:])
            # reduce + transpose to columns: cat4[m, s, kind] (kind: a=0, g=1)
            psum_ag = ph.tile([P, 2, 2], F32)
            nc.tensor.matmul(
                psum_ag[:].rearrange("m s k -> m (s k)"),
                g_sb[:], pkf_t[0:krows, SELA:SELA + 4], start=True, stop=True)
            cat4 = sb.tile([P, 2, 2], F32)
            nc.scalar.copy(cat4[:], psum_ag[:])
            a_v = cat4[:, :, 0]                       # [128, 2] a state (fp32)
            g_v = cat4[:, :, 1]                       # [128, 2] gv columns
            neg01_bf = sb.tile([1, P], BF16)
            nc.vector.memset(neg01_bf[:], -0.1)

            # ----------------------------------------------- phase A: 10 steps
            # unconditional recurrence; per-step logits land in psum3_all
            psum3_all = ph.tile([1, MAX_STEPS, 2], F32)
            b3sp = pkf_t[0:1, B3C:B3C + 1]
            b3st = pkf_t[0:1, B3C + 1:B3C + 2]
            for t in range(MAX_STEPS):
                u = sb.tile([P, 2], BF16, tag="u")
                nc.vector.tensor_scalar_max(u[:], a_v, 0.0)

                p2 = ps.tile([P, 2], F32, tag="small")
                nc.tensor.matmul(p2[:], pkb_t[:, W2B:W2B + P], u[:],
                                 start=True, stop=True)
                # h2z halves: relu(p2 + b2); off-diagonal halves stay zero
                nc.scalar.activation(
                    h2z[0:64, 0:1], p2[0:64, 0:1],
                    mybir.ActivationFunctionType.Relu,
                    bias=pkf_t[0:64, B2C:B2C + 1],
                )
                nc.vector.scalar_tensor_tensor(
                    h2z[64:128, 1:2], p2[64:128, 1:2],
                    pkf_t[64:128, B2C:B2C + 1], zcol[64:128, 0:1],
                    op0=mybir.AluOpType.add, op1=mybir.AluOpType.max,
                )

                nc.tensor.matmul(psum3_all[:, t, :], pkb_t[:, W3B:W3B + 1],
                                 h2z[:], start=True, stop=True)

                sig_bf = sb.tile([1, 1], BF16, tag="sig")
                nc.scalar.activation(
                    sig_bf[:], psum3_all[:, t, 0:1],
                    mybir.ActivationFunctionType.Sigmoid, bias=b3sp,
                )
                # a += (-0.1*ss) * gv  (scalar broadcast via 1x128 matmul)
                pb = ps.tile([P, 1], F32, tag="small")
                nc.tensor.matmul(pb[:], neg01_bf[:], sig_bf[:],
                                 start=True, stop=True)
                nc.vector.scalar_tensor_tensor(
                    a_v, g_v, pb[:], a_v,
                    op0=mybir.AluOpType.mult, op1=mybir.AluOpType.add,
                )

            # ----------------------------------------------- phase B: gating
            ss_row = sb.tile([1, MAX_STEPS], F32)
            nc.scalar.activation(ss_row[:], psum3_all[:, :, 0],
                                 mybir.ActivationFunctionType.Sigmoid, bias=b3sp)
            pstop_row = sb.tile([1, MAX_STEPS], F32)
            nc.scalar.activation(pstop_row[:], psum3_all[:, :, 1],
                                 mybir.ActivationFunctionType.Sigmoid, bias=b3st)
            notstop = sb.tile([1, MAX_STEPS], F32)
            nc.vector.tensor_tensor(
                notstop[:], pkf_t[0:1, RVC:RVC + MAX_STEPS], pstop_row[:],
                op=mybir.AluOpType.is_ge,
            )
            # act_t = prod_{tau<t} notstop_tau (tensor_tensor_scan and
            # tensor_tensor_reduce both crash the exec unit on this HW
            # config, so use plain serial DVE ops — it's only 10 steps)
            act_row = sb.tile([1, MAX_STEPS], F32)
            nc.vector.memset(act_row[:, 0:1], 1.0)
            for t in range(1, MAX_STEPS):
                nc.vector.tensor_tensor(
                    act_row[:, t:t + 1], act_row[:, t - 1:t],
                    notstop[:, t - 1:t], op=mybir.AluOpType.mult,
                )
            # S_neg = -0.1 * sum(act*ss); count = sum(act)
            sprod = sb.tile([1, MAX_STEPS], F32)
            nc.vector.tensor_tensor(sprod[:], act_row[:], ss_row[:],
                                    op=mybir.AluOpType.mult)
            ssum = sb.tile([1, 1], F32)
            nc.vector.tensor_reduce(ssum[:], sprod[:],
                                    axis=mybir.AxisListType.X,
                                    op=mybir.AluOpType.add)
            s_neg = sb.tile([1, 1], F32)
            nc.scalar.mul(s_neg[:], ssum[:], -0.1)
            cnt_f = sb.tile([1, 1], F32)
            nc.vector.tensor_reduce(cnt_f[:], act_row[:],
                                    axis=mybir.AxisListType.X,
                                    op=mybir.AluOpType.add)
            cnt_i = sb.tile([1, 1], I32)
            nc.vector.tensor_copy(cnt_i[:], cnt_f[:])
            nc.sync.dma_start(out_count[:], cnt_i[:])

            # ----------------------------------------------- final update
            pS = ps.tile([P, 1], F32, tag="small")
            nc.tensor.matmul(pS[:], ones_row[:], s_neg[:], start=True, stop=True)
            out_t = sb.tile([P, C], F32)
            nc.vector.scalar_tensor_tensor(
                out_t[:], pkf_t[:, XPG:XPG + C], pS[:], pkf_t[:, XPP:XPP + C],
                op0=mybir.AluOpType.mult, op1=mybir.AluOpType.add,
            )
            nc.sync.dma_start(out_params[:], out_t[:])

    nc.compile()
    return nc


_COMPILED = {}


def _get_program(collective=COLLECTIVE):
    if collective not in _COMPILED:
        _COMPILED[collective] = _build_program(collective)
    return _COMPILED[collective]


def _prep_inputs(inputs):
    """Build the 8 per-core input maps from full-size numpy inputs."""
    f = lambda k: np.asarray(inputs[k], dtype=np.float32)
    p0, g = f("initial_params"), f("gradients")
    sp_w1, st_w1 = f("sp_w1"), f("st_w1")

    # ---- shared packed fp32 tensor
    pkf = np.zeros((P, PKF_COLS), np.float32)
    pkf[0:64, B2C] = f("sp_b2")
    pkf[64:128, B2C] = f("st_b2")
    pkf[0:CTX, CW1:CW1 + 64] = f("ce_w1").T
    pkf[0:64, CW2:CW2 + 64] = f("ce_w2").T
    pkf[0:CTX, CTXC] = f("context")
    pkf[0:64, CB1C] = f("ce_b1")
    pkf[0:64, CB2C] = f("ce_b2")
    pkf[0, B3C] = f("sp_b3")[0]
    pkf[0, B3C + 1] = f("st_b3")[0]
    rv = f("rand_vals").copy()
    rv[0] = 2.0  # MIN_STEPS=1: step 0 can never stop
    pkf[0, RVC:RVC + MAX_STEPS] = rv
    # cc rows (per rank) are 2*kind+s; cat4 column n = 2*s+kind
    for k in range(4 * NCORES):
        kind, s = (k % 4) // 2, (k % 4) % 2
        pkf[k, SELA + 2 * s + kind] = 1.0
    pkf_shared = pkf

    # ---- shared part of packed bf16 tensor (w2cat/w3cat/tail)
    pkb_shared = np.zeros((P, PKB_COLS), BF)
    pkb_shared[:, W2B:W2B + P] = np.concatenate(
        [f("sp_w2").T, f("st_w2").T], axis=1).astype(BF)
    pkb_shared[0:64, W3B] = f("sp_w3")[0].astype(BF)
    pkb_shared[64:128, W3B] = f("st_w3")[0].astype(BF)
    tail_sp = np.concatenate(
        [sp_w1[:, N:].T, f("sp_b1")[None, :]], axis=0) / NCORES  # [65,128]
    tail_st = np.concatenate(
        [st_w1[:, N:].T, f("st_b1")[None, :]], axis=0) / NCORES
    pkb_shared[0:65, TLB:TLB + 2 * P] = np.concatenate(
        [tail_sp, tail_st], axis=1).astype(BF)

    in_maps = []
    for r in range(NCORES):
        sl = slice(r * NS, (r + 1) * NS)
        # wt[k, (c, s, m)] = W_s[m, r*NS + c*128 + k]
        ws = np.zeros((2, P, NSP), np.float32)
        ws[0, :, :NS] = sp_w1[:, sl]
        ws[1, :, :NS] = st_w1[:, sl]
        wt = np.ascontiguousarray(
            ws.reshape(2, P, C, P).transpose(3, 2, 0, 1)).reshape(P, WCOLS)

        p_pad = np.zeros(NSP, np.float32)
        p_pad[:NS] = p0[sl]
        g_pad = np.zeros(NSP, np.float32)
        g_pad[:NS] = g[sl]

        pkb_r = pkb_shared.copy()
        pkb_r[:, XB:XB + 2 * C] = np.stack(
            [p_pad.reshape(C, P).T, g_pad.reshape(C, P).T], axis=2
        ).reshape(P, 2 * C).astype(BF)

        pkf_r = pkf_shared.copy()
        pkf_r[:, XPP:XPP + C] = p_pad.reshape(C, P).T
        pkf_r[:, XPG:XPG + C] = g_pad.reshape(C, P).T

        in_maps.append({
            "wt": wt.astype(BF),
            "pkb": pkb_r,
            "pkf": pkf_r,
        })
    return in_maps


LAST_RESULTS = None  # BassKernelResults of the most recent run (for test.py)


def kernel(trace=False, collective=COLLECTIVE, **inputs):
    global LAST_RESULTS
    nc = _get_program(collective)
    in_maps = _prep_inputs(inputs)
    res = run_bass_kernel_spmd(
        nc, in_maps, list(range(NCORES)), trace=trace,
    )
    LAST_RESULTS = res
    outs = res.results
    params = np.empty(N, np.float32)
    for r in range(NCORES):
        shard = outs[r]["out_params"]  # [128, 196], (k, c) = p[c*128+k]
        params[r * NS:(r + 1) * NS] = shard.T.reshape(-1)[:NS]
    count = np.int32(outs[0]["out_count"].reshape(-1)[0])
    return params, count
